# revision 12
# baseline (speedup 1.0000x reference)
"""Trainium2 Bass kernel for nn_ContrastLoss.

Reference computation (B=128, P=256 proposals/image, D=1024, K=4 scales):
    box_n = l2norm(box.reshape(B,P,D));  z_n = l2norm(crop)      # [K,B,D]
    cos   = einsum('bpd,kbd->kbp', box_n, z_n)
    mask  = ious >= 0.4  (per (b,p));  cnt_pos = mask.sum(p)
    sim_pos = -(cos*mask).sum(p)/cnt_pos ; sim_neg = -(cos*~mask).sum(p)/cnt_neg
    L[k] = softplus((sim_neg-sim_pos)/T).sum(b);  out = min_k L / B

Key algebraic restructure (per batch b):
    arg[k,b] = (sim_neg-sim_pos)/T = (z[k,b] . S[b]) / ||z[k,b]||
    S[b,d]   = sum_p w[b,p] * box[b,p,d]
    w[b,p]   = invnorm[b,p] * (mask*(1/cnt_pos+1/cnt_neg) - 1/cnt_neg)/T
so the only heavy pass over the 128 MiB box tensor is one streaming read that
feeds (a) a row-wise sum-of-squares (ScalarE, fused accumulate) and (b) a
PE matmul contraction over proposals with sparse [128,16] weight columns.

Pipeline layout (per core, 16 MiB box slice):
  - 16 box chunks of 2 row-tiles (1 MiB) streamed on the SP DMA queue;
    small inputs (iou/crop/rep) go on the Pool DMA queue so they don't
    delay the first chunk.
  - per chunk: 2 ACT squares (fused row sum-of-squares) -> DVE reciprocal
    -> ACT sqrt -> one DVE multiply writing both weight columns directly
    into the sparse lhsT layout (strided, no scatter pass) -> 4 PE matmuls
    accumulating S in PSUM.
  - final: copy S to SBUF (split ACT/DVE), replicate S across K scales with
    one small PE matmul (host-supplied 0/1 rep matrix), one fused DVE
    tensor_tensor_reduce for all 64 (k,b) dots, scale by 1/||z||, DMA out.
Host applies softplus, sums across cores, takes min over k and divides by B.
"""

import contextlib
import sys

if "/opt/trn_rl_repo" not in sys.path:
    sys.path.insert(0, "/opt/trn_rl_repo")

import numpy as np

import concourse.bacc as bacc
import concourse.mybir as mybir
import concourse.tile as tile
from concourse.bass_utils import run_bass_kernel_spmd

# Problem constants (hardcoded per harness contract).
B, P, D, K = 128, 256, 1024, 4
N_CORES = 8
B_CORE = B // N_CORES            # 16 batches per core
ROWS = B_CORE * P                # 4096 rows per core
NT = ROWS // 128                 # 32 row-tiles of 128 rows
CH_TILES = 2                     # row-tiles per DMA chunk
N_CHUNKS = NT // CH_TILES        # 16 chunks of 1 MiB
KB = K * B_CORE                  # 64 (k, b) pairs per core
IOU_THRES = 0.4
TEMP = 0.2

F32 = mybir.dt.float32
F32R = mybir.dt.float32r
BF16 = mybir.dt.bfloat16
AF = mybir.ActivationFunctionType
ALU = mybir.AluOpType


def _emit(tc):
    nc = tc.nc
    box = nc.dram_tensor("box", [ROWS, D], F32, kind="ExternalInput").ap()
    iou_t = nc.dram_tensor("iou_t", [128, NT], F32, kind="ExternalInput").ap()
    zflat = nc.dram_tensor("zflat", [KB, D], F32, kind="ExternalInput").ap()
    rep = nc.dram_tensor("rep", [B_CORE, KB], F32, kind="ExternalInput").ap()
    out_l = nc.dram_tensor("out_l", [KB, 1], F32, kind="ExternalOutput").ap()

    ctx = contextlib.ExitStack()
    with ctx:
        const = ctx.enter_context(tc.tile_pool(name="const", bufs=1))
        boxpool = ctx.enter_context(tc.tile_pool(name="boxpool", bufs=N_CHUNKS))
        sqpool = ctx.enter_context(tc.tile_pool(name="sqpool", bufs=2))
        psS = ctx.enter_context(tc.tile_pool(name="psS", bufs=1, space="PSUM"))
        psDot = ctx.enter_context(tc.tile_pool(name="psDot", bufs=1, space="PSUM"))
        psmisc = ctx.enter_context(tc.tile_pool(name="psmisc", bufs=1, space="PSUM"))

        # --- small inputs first (SP queue; cheap descriptor gens) ---------
        iou_sb = const.tile([128, NT], F32)
        nc.sync.dma_start(iou_sb[:], iou_t[:])
        z_sb = const.tile([KB, D], F32)
        nc.sync.dma_start(z_sb[:], zflat[:])
        rep_sb = const.tile([B_CORE, KB], F32R)
        nc.sync.dma_start(rep_sb[:], rep[:].bitcast(F32R))

        # --- box chunk DMAs (streaming, SP queue) -------------------------
        CH_COLS = CH_TILES * D
        box3 = box.rearrange("(t p) d -> p t d", p=128)
        chunks = []
        for c in range(N_CHUNKS):
            # f32r tiles: the walrus verifier requires every producer of an
            # fp32r matmul operand to itself emit f32r (pre-rounded).
            ch = boxpool.tile([128, CH_COLS], F32R, name=f"ch{c}", tag="ch")
            ch3 = ch.rearrange("p (t d) -> p t d", d=D)
            src = box3[:, c * CH_TILES:(c + 1) * CH_TILES, :].bitcast(F32R)
            nc.sync.dma_start(ch3, src)
            chunks.append(ch)

        # sparse per-tile weight columns: w_sp[:, 16*t + t//2] nonzero.
        # f32r (fp32r matmul producer rule); Memset cannot emit f32r, so
        # zero it via a DVE copy from a Pool-memset f32 tile.
        w_sp = const.tile([128, NT * B_CORE], F32R)
        zw = const.tile([128, NT * B_CORE], F32)
        nc.vector.memset(zw[:], 0.0)
        nc.vector.tensor_copy(w_sp[:], zw[:])

        # --- mask / count / coefficient setup (DVE + tiny PE) -------------
        # bf16 for the tiny count/broadcast matmuls: walrus codegen rejects
        # the fp32 lowering of K=1/M=1 matmuls, and bf16 is exact for
        # ones/0-1 masks while coef rounding (~4e-3) is far below tolerance.
        ones_col = const.tile([128, 1], BF16)
        nc.vector.memset(ones_col[:], 1.0)
        ones_row = const.tile([1, 128], BF16)
        nc.vector.memset(ones_row[:], 1.0)

        # mask[p, t] = iou >= thres  (1.0 / 0.0)
        mask = const.tile([128, NT], BF16)
        nc.vector.tensor_scalar(mask[:], iou_sb[:], IOU_THRES, None, ALU.is_ge)

        # cnt per row-tile column: ones[128,1].T @ mask -> [1, NT]
        ps_cnt = psmisc.tile([1, NT], F32)
        nc.tensor.matmul(ps_cnt[:], ones_col[:], mask[:], start=True, stop=True)

        cnt_t = const.tile([1, NT], F32)
        nc.vector.tensor_copy(cnt_t[:], ps_cnt[:])
        cnt_pos = const.tile([1, B_CORE], F32)
        nc.vector.tensor_tensor(
            cnt_pos[:], cnt_t[0:1, 0:NT:2], cnt_t[0:1, 1:NT:2], ALU.add
        )
        rcp_p = const.tile([1, B_CORE], F32)
        nc.vector.reciprocal(rcp_p[:], cnt_pos[:])
        cnt_neg = const.tile([1, B_CORE], F32)
        nc.vector.tensor_scalar(
            cnt_neg[:], cnt_pos[:], -1.0, float(P), ALU.mult, ALU.add
        )
        rcp_n = const.tile([1, B_CORE], F32)
        nc.vector.reciprocal(rcp_n[:], cnt_neg[:])

        # coefA=(rcp_p+rcp_n)/T at cols 2b,2b+1 ; coefB=rcp_n/T at NT+...
        coef_row = const.tile([1, 2 * NT], BF16)
        tmp_ab = const.tile([1, B_CORE], F32)
        nc.vector.tensor_tensor(tmp_ab[:], rcp_p[:], rcp_n[:], ALU.add)
        for r in range(2):
            nc.vector.tensor_scalar(
                coef_row[0:1, r:NT:2], tmp_ab[:], 1.0 / TEMP, None, ALU.mult
            )
            nc.vector.tensor_scalar(
                coef_row[0:1, NT + r:2 * NT:2], rcp_n[:], 1.0 / TEMP,
                None, ALU.mult,
            )

        # broadcast to all 128 partitions: ones[1,128].T @ coef[1,2NT]
        ps_coef = psmisc.tile([128, 2 * NT], F32)
        nc.tensor.matmul(ps_coef[:], ones_row[:], coef_row[:], start=True, stop=True)
        coef_bc = const.tile([128, 2 * NT], F32)
        nc.vector.tensor_copy(coef_bc[:], ps_coef[:])

        # pre_w[:, t] = mask*coefA - coefB (invnorm applied per chunk later)
        pre_w = const.tile([128, NT], F32)
        nc.vector.tensor_tensor(pre_w[:], mask[:], coef_bc[:, 0:NT], ALU.mult)
        nc.vector.tensor_tensor(
            pre_w[:], pre_w[:], coef_bc[:, NT:2 * NT], ALU.subtract
        )

        # --- z inverse norms (independent of the box stream) --------------
        zss = const.tile([KB, 1], F32)
        zsq = const.tile([KB, D], F32)
        nc.scalar.activation(zsq[:], z_sb[:], AF.Square, accum_out=zss[:])
        zrec = const.tile([KB, 1], F32)
        nc.vector.reciprocal(zrec[:], zss[:])
        zinv = const.tile([KB, 1], F32)
        nc.scalar.activation(zinv[:], zrec[:], AF.Sqrt)

        # --- main streaming pass over box ---------------------------------
        ps_S = psS.tile([B_CORE, D], F32)
        ss_all = const.tile([128, NT], F32)
        rec_all = const.tile([128, NT], F32)
        invn = const.tile([128, NT], F32)

        for c in range(N_CHUNKS):
            ch = chunks[c]
            t0 = c * CH_TILES
            for rt in range(CH_TILES):
                t = t0 + rt
                sq = sqpool.tile([128, D], F32, name="sq", tag="sq")
                nc.scalar.activation(
                    sq[:], ch[:, rt * D:(rt + 1) * D].bitcast(F32), AF.Square,
                    accum_out=ss_all[:, t:t + 1],
                )
            nc.vector.reciprocal(
                rec_all[:, t0:t0 + CH_TILES], ss_all[:, t0:t0 + CH_TILES]
            )
            nc.scalar.activation(
                invn[:, t0:t0 + CH_TILES], rec_all[:, t0:t0 + CH_TILES], AF.Sqrt
            )
            # weight cols for tiles (2c, 2c+1) live at 33c and 33c+16
            a = 33 * c
            nc.vector.tensor_tensor(
                w_sp[:, a:a + 17:16], pre_w[:, t0:t0 + CH_TILES],
                invn[:, t0:t0 + CH_TILES], ALU.mult,
            )
            for rt in range(CH_TILES):
                t = t0 + rt
                lhsT = w_sp[:, t * B_CORE:(t + 1) * B_CORE]
                for h in range(2):
                    nc.tensor.matmul(
                        ps_S[:, h * 512:(h + 1) * 512],
                        lhsT,
                        ch[:, rt * D + h * 512:rt * D + (h + 1) * 512],
                        start=(t == 0),
                        stop=(t == NT - 1),
                        skip_group_check=True,
                    )

        # --- final dots: args[k*16+b] = (z[k,b] . S[b]) * zinv ------------
        s_sb = const.tile([B_CORE, D], F32R)
        nc.vector.tensor_copy(s_sb[:], ps_S[:])

        ps_dot = psDot.tile([KB, D], F32)
        for h in range(2):
            nc.tensor.matmul(
                ps_dot[:, h * 512:(h + 1) * 512],
                rep_sb[:],
                s_sb[:, h * 512:(h + 1) * 512],
                start=True,
                stop=True,
            )

        prod = const.tile([KB, D], F32)
        dots = const.tile([KB, 1], F32)
        nc.vector.tensor_tensor(prod[:], z_sb[:], ps_dot[:], ALU.mult)
        nc.vector.reduce_sum(dots[:], prod[:], axis=mybir.AxisListType.X)
        args = const.tile([KB, 1], F32)
        nc.vector.tensor_scalar(args[:], dots[:], zinv[:], None, ALU.mult)
        # softplus + batch-sum + min over k happen on the host (512 scalars)
        nc.sync.dma_start(out_l[:], args[:])


_NC_CACHE = None


def _get_nc():
    global _NC_CACHE
    if _NC_CACHE is None:
        nc = bacc.Bacc(
            "TRN2", target_bir_lowering=False, debug=False, num_devices=N_CORES
        )
        with tile.TileContext(nc) as tc:
            _emit(tc)
        nc.compile()
        _NC_CACHE = nc
    return _NC_CACHE


def _in_maps(box_cls_feat_con, crop_feat_con, ious):
    box = np.ascontiguousarray(np.asarray(box_cls_feat_con, dtype=np.float32))
    crop = np.ascontiguousarray(np.asarray(crop_feat_con, dtype=np.float32))
    iou = np.asarray(ious, dtype=np.float32)
    rep = np.zeros((B_CORE, KB), dtype=np.float32)
    for k in range(K):
        for b in range(B_CORE):
            rep[b, k * B_CORE + b] = 1.0
    maps = []
    for c in range(N_CORES):
        rows = slice(c * ROWS, (c + 1) * ROWS)
        bsl = slice(c * B_CORE, (c + 1) * B_CORE)
        maps.append({
            "box": np.ascontiguousarray(box[rows]),
            "iou_t": np.ascontiguousarray(iou[rows].reshape(NT, 128).T),
            "zflat": np.ascontiguousarray(
                crop[:, bsl, :].reshape(KB, D)
            ),
            "rep": rep,
        })
    return maps


def kernel(box_cls_feat_con, crop_feat_con, batch_size, ious, _trace=False):
    nc = _get_nc()
    maps = _in_maps(box_cls_feat_con, crop_feat_con, ious)
    res = run_bass_kernel_spmd(nc, maps, core_ids=list(range(N_CORES)), trace=_trace)
    l_total = np.zeros(K, dtype=np.float64)
    for c in range(N_CORES):
        args = res.results[c]["out_l"].astype(np.float64).reshape(K, B_CORE)
        l_total += np.log1p(np.exp(args)).sum(axis=1)
    out = np.float32(l_total.min() / float(B))
    if _trace:
        kernel._last_results = res
    return np.asarray(out, dtype=np.float32)


# revision 14
# speedup vs baseline: 1.0554x; 1.0554x over previous
"""Trainium2 Bass kernel for nn_ContrastLoss.

Reference computation (B=128, P=256 proposals/image, D=1024, K=4 scales):
    box_n = l2norm(box.reshape(B,P,D));  z_n = l2norm(crop)      # [K,B,D]
    cos   = einsum('bpd,kbd->kbp', box_n, z_n)
    mask  = ious >= 0.4  (per (b,p));  cnt_pos = mask.sum(p)
    sim_pos = -(cos*mask).sum(p)/cnt_pos ; sim_neg = -(cos*~mask).sum(p)/cnt_neg
    L[k] = softplus((sim_neg-sim_pos)/T).sum(b);  out = min_k L / B

Key algebraic restructure (per batch b):
    arg[k,b] = (sim_neg-sim_pos)/T = (z[k,b] . S[b]) / ||z[k,b]||
    S[b,d]   = sum_p w[b,p] * box[b,p,d]
    w[b,p]   = invnorm[b,p] * (mask*(1/cnt_pos+1/cnt_neg) - 1/cnt_neg)/T
so the only heavy pass over the 128 MiB box tensor is one streaming read that
feeds (a) a row-wise sum-of-squares (ScalarE, fused accumulate) and (b) a
PE matmul contraction over proposals.

The matmul lhsT for row-tile t (all 128 rows belong to batch b=t//2) carries
the weight column replicated at cols k*16+b for k=0..3, so PSUM directly
accumulates rep_S[(k,b), d] = S[b, d] in the [64, D] layout the final dot
needs -- no post-stream S copy or replication matmul.

Pipeline layout (per core, 16 MiB box slice):
  - iou + crop DMAs, then 15x 1-MiB + 2x 0.5-MiB box chunks, all on the SP
    queue; the stream is the DMA-roofline critical path (~47 us).
  - per chunk: ACT squares (fused row sum-of-squares) -> DVE reciprocal ->
    ACT sqrt -> one broadcast DVE multiply writing all replicated weight
    columns -> PE matmuls accumulating rep_S in PSUM.
  - final: one fused DVE tensor_tensor_reduce over [64, D] for all (k,b)
    dots, scale by 1/||z|| (per-partition scalar), DMA out.
Host applies softplus, sums across cores, takes min over k and divides by B.
"""

import contextlib
import sys

if "/opt/trn_rl_repo" not in sys.path:
    sys.path.insert(0, "/opt/trn_rl_repo")

import numpy as np

import concourse.bacc as bacc
import concourse.mybir as mybir
import concourse.tile as tile
from concourse.bass_utils import run_bass_kernel_spmd

# Problem constants (hardcoded per harness contract).
B, P, D, K = 128, 256, 1024, 4
N_CORES = 8
B_CORE = B // N_CORES            # 16 batches per core
ROWS = B_CORE * P                # 4096 rows per core
NT = ROWS // 128                 # 32 row-tiles of 128 rows
KB = K * B_CORE                  # 64 (k, b) pairs per core
CH_SIZES = [2] * 15 + [1, 1]     # row-tiles per chunk (small tail chunks)
IOU_THRES = 0.4
TEMP = 0.2

F32 = mybir.dt.float32
F32R = mybir.dt.float32r
BF16 = mybir.dt.bfloat16
AF = mybir.ActivationFunctionType
ALU = mybir.AluOpType


def _emit(tc):
    nc = tc.nc
    box = nc.dram_tensor("box", [ROWS, D], F32, kind="ExternalInput").ap()
    iou_t = nc.dram_tensor("iou_t", [128, NT], F32, kind="ExternalInput").ap()
    zflat = nc.dram_tensor("zflat", [KB, D], F32, kind="ExternalInput").ap()
    out_l = nc.dram_tensor("out_l", [KB, 1], F32, kind="ExternalOutput").ap()

    ctx = contextlib.ExitStack()
    with ctx:
        const = ctx.enter_context(tc.tile_pool(name="const", bufs=1))
        boxpool = ctx.enter_context(tc.tile_pool(name="boxpool", bufs=len(CH_SIZES)))
        sqpool = ctx.enter_context(tc.tile_pool(name="sqpool", bufs=2))
        psDot = ctx.enter_context(tc.tile_pool(name="psDot", bufs=1, space="PSUM"))
        psmisc = ctx.enter_context(tc.tile_pool(name="psmisc", bufs=1, space="PSUM"))

        # --- input DMAs on the SP queue: iou + crop, then the box stream --
        iou_sb = const.tile([128, NT], F32)
        nc.sync.dma_start(iou_sb[:], iou_t[:])
        z_sb = const.tile([KB, D], F32)
        nc.sync.dma_start(z_sb[:], zflat[:])

        box3 = box.rearrange("(t p) d -> p t d", p=128)
        chunks = []
        t0 = 0
        for c, n_t in enumerate(CH_SIZES):
            # f32r tiles: the walrus verifier requires every producer of an
            # fp32r matmul operand to itself emit f32r (pre-rounded).
            ch = boxpool.tile([128, n_t * D], F32R, name=f"ch{c}", tag="ch")
            ch3 = ch.rearrange("p (t d) -> p t d", d=D)
            src = box3[:, t0:t0 + n_t, :].bitcast(F32R)
            nc.sync.dma_start(ch3, src)
            chunks.append((ch, t0, n_t))
            t0 += n_t
        assert t0 == NT

        # --- ACT function-table preloads (Square + Sqrt) at t~0 -----------
        dumm = const.tile([1, 1], F32)
        nc.vector.memset(dumm[:], 1.0)
        dummo = const.tile([1, 1], F32)
        nc.scalar.activation(dummo[:], dumm[:], AF.Square)
        nc.scalar.activation(dummo[:], dumm[:], AF.Sqrt)

        # replicated sparse weight columns: for row-tile t (batch b=t//2)
        # cols t*64 + k*16 + b, k=0..3, hold the weight column; rest zero.
        # f32r (fp32r matmul producer rule); Memset cannot emit f32r, so
        # zero it via a DVE copy from a memset f32 tile.
        w2 = const.tile([128, NT * KB], F32R)
        zw = const.tile([128, NT * KB], F32)
        nc.vector.memset(zw[:], 0.0)
        nc.vector.tensor_copy(w2[:], zw[:])
        w2v = w2.rearrange("p (t k s) -> p t k s", k=K, s=B_CORE)

        # --- mask / count / coefficient setup (DVE + tiny PE) -------------
        # bf16 for the tiny count/broadcast matmuls: walrus codegen rejects
        # the fp32 lowering of K=1/M=1 matmuls, and bf16 is exact for
        # ones/0-1 masks while coef rounding (~4e-3) is far below tolerance.
        ones_col = const.tile([128, 1], BF16)
        nc.vector.memset(ones_col[:], 1.0)
        ones_row = const.tile([1, 128], BF16)
        nc.vector.memset(ones_row[:], 1.0)

        # mask[p, t] = iou >= thres  (1.0 / 0.0)
        mask = const.tile([128, NT], BF16)
        nc.vector.tensor_scalar(mask[:], iou_sb[:], IOU_THRES, None, ALU.is_ge)

        # cnt per row-tile column: ones[128,1].T @ mask -> [1, NT]
        ps_cnt = psmisc.tile([1, NT], F32)
        nc.tensor.matmul(ps_cnt[:], ones_col[:], mask[:], start=True, stop=True)

        cnt_t = const.tile([1, NT], F32)
        nc.vector.tensor_copy(cnt_t[:], ps_cnt[:])
        cnt_pos = const.tile([1, B_CORE], F32)
        nc.vector.tensor_tensor(
            cnt_pos[:], cnt_t[0:1, 0:NT:2], cnt_t[0:1, 1:NT:2], ALU.add
        )
        rcp_p = const.tile([1, B_CORE], F32)
        nc.vector.reciprocal(rcp_p[:], cnt_pos[:])
        cnt_neg = const.tile([1, B_CORE], F32)
        nc.vector.tensor_scalar(
            cnt_neg[:], cnt_pos[:], -1.0, float(P), ALU.mult, ALU.add
        )
        rcp_n = const.tile([1, B_CORE], F32)
        nc.vector.reciprocal(rcp_n[:], cnt_neg[:])

        # coefA=(rcp_p+rcp_n)/T at cols 2b,2b+1 ; coefB=rcp_n/T at NT+...
        coef_row = const.tile([1, 2 * NT], BF16)
        tmp_ab = const.tile([1, B_CORE], F32)
        nc.vector.tensor_tensor(tmp_ab[:], rcp_p[:], rcp_n[:], ALU.add)
        for r in range(2):
            nc.vector.tensor_scalar(
                coef_row[0:1, r:NT:2], tmp_ab[:], 1.0 / TEMP, None, ALU.mult
            )
            nc.vector.tensor_scalar(
                coef_row[0:1, NT + r:2 * NT:2], rcp_n[:], 1.0 / TEMP,
                None, ALU.mult,
            )

        # broadcast to all 128 partitions: ones[1,128].T @ coef[1,2NT]
        ps_coef = psmisc.tile([128, 2 * NT], F32)
        nc.tensor.matmul(ps_coef[:], ones_row[:], coef_row[:], start=True, stop=True)
        coef_bc = const.tile([128, 2 * NT], F32)
        nc.vector.tensor_copy(coef_bc[:], ps_coef[:])

        # pre_w[:, t] = mask*coefA - coefB (invnorm applied per chunk later)
        pre_w = const.tile([128, NT], F32)
        nc.vector.tensor_tensor(pre_w[:], mask[:], coef_bc[:, 0:NT], ALU.mult)
        nc.vector.tensor_tensor(
            pre_w[:], pre_w[:], coef_bc[:, NT:2 * NT], ALU.subtract
        )

        # --- z inverse norms (independent of the box stream) --------------
        zss = const.tile([KB, 1], F32)
        zsq = const.tile([KB, D], F32)
        nc.scalar.activation(zsq[:], z_sb[:], AF.Square, accum_out=zss[:])
        zrec = const.tile([KB, 1], F32)
        nc.vector.reciprocal(zrec[:], zss[:])
        zinv = const.tile([KB, 1], F32)
        nc.scalar.activation(zinv[:], zrec[:], AF.Sqrt)

        # --- main streaming pass over box ---------------------------------
        ps_dot = psDot.tile([KB, D], F32)
        ss_all = const.tile([128, NT], F32)
        rec_all = const.tile([128, NT], F32)
        invn = const.tile([128, NT], F32)

        for ch, t0, n_t in chunks:
            for rt in range(n_t):
                t = t0 + rt
                sq = sqpool.tile([128, D], F32, name="sq", tag="sq")
                nc.scalar.activation(
                    sq[:], ch[:, rt * D:(rt + 1) * D].bitcast(F32), AF.Square,
                    accum_out=ss_all[:, t:t + 1],
                )
            nc.vector.reciprocal(
                rec_all[:, t0:t0 + n_t], ss_all[:, t0:t0 + n_t]
            )
            nc.scalar.activation(
                invn[:, t0:t0 + n_t], rec_all[:, t0:t0 + n_t], AF.Sqrt
            )
            # all tiles in a chunk share batch pairing b=t//2; broadcast the
            # weight column into the K replicated sparse positions at once
            b0 = t0 // 2
            dst = w2v[:, t0:t0 + n_t, :, b0]
            pw = pre_w[:, t0:t0 + n_t].rearrange(
                "p (t one) -> p t one", one=1
            ).broadcast_to([128, n_t, K])
            iv = invn[:, t0:t0 + n_t].rearrange(
                "p (t one) -> p t one", one=1
            ).broadcast_to([128, n_t, K])
            nc.vector.tensor_tensor(dst, pw, iv, ALU.mult)
            for rt in range(n_t):
                t = t0 + rt
                lhsT = w2[:, t * KB:(t + 1) * KB]
                for h in range(2):
                    nc.tensor.matmul(
                        ps_dot[:, h * 512:(h + 1) * 512],
                        lhsT,
                        ch[:, rt * D + h * 512:rt * D + (h + 1) * 512],
                        start=(t == 0),
                        stop=(t == NT - 1),
                        skip_group_check=True,
                    )

        # --- final dots: args[k*16+b] = (z[k,b] . S[b]) * zinv ------------
        # (tensor_tensor_reduce would fuse these, but it faults at runtime
        # under this PJRT path -- CoreSim-only support.)
        prod = const.tile([KB, D], F32)
        dots = const.tile([KB, 1], F32)
        nc.vector.tensor_tensor(prod[:], z_sb[:], ps_dot[:], ALU.mult)
        nc.vector.reduce_sum(dots[:], prod[:], axis=mybir.AxisListType.X)
        args = const.tile([KB, 1], F32)
        nc.vector.tensor_scalar(args[:], dots[:], zinv[:], None, ALU.mult)
        # softplus + batch-sum + min over k happen on the host (512 scalars)
        nc.sync.dma_start(out_l[:], args[:])


_NC_CACHE = None


def _get_nc():
    global _NC_CACHE
    if _NC_CACHE is None:
        nc = bacc.Bacc(
            "TRN2", target_bir_lowering=False, debug=False, num_devices=N_CORES
        )
        with tile.TileContext(nc) as tc:
            _emit(tc)
        nc.compile()
        _NC_CACHE = nc
    return _NC_CACHE


def _in_maps(box_cls_feat_con, crop_feat_con, ious):
    box = np.ascontiguousarray(np.asarray(box_cls_feat_con, dtype=np.float32))
    crop = np.ascontiguousarray(np.asarray(crop_feat_con, dtype=np.float32))
    iou = np.asarray(ious, dtype=np.float32)
    maps = []
    for c in range(N_CORES):
        rows = slice(c * ROWS, (c + 1) * ROWS)
        bsl = slice(c * B_CORE, (c + 1) * B_CORE)
        maps.append({
            "box": np.ascontiguousarray(box[rows]),
            "iou_t": np.ascontiguousarray(iou[rows].reshape(NT, 128).T),
            "zflat": np.ascontiguousarray(crop[:, bsl, :].reshape(KB, D)),
        })
    return maps


def kernel(box_cls_feat_con, crop_feat_con, batch_size, ious, _trace=False):
    nc = _get_nc()
    maps = _in_maps(box_cls_feat_con, crop_feat_con, ious)
    res = run_bass_kernel_spmd(nc, maps, core_ids=list(range(N_CORES)), trace=_trace)
    l_total = np.zeros(K, dtype=np.float64)
    for c in range(N_CORES):
        args = res.results[c]["out_l"].astype(np.float64).reshape(K, B_CORE)
        l_total += np.log1p(np.exp(args)).sum(axis=1)
    out = np.float32(l_total.min() / float(B))
    if _trace:
        kernel._last_results = res
    return np.asarray(out, dtype=np.float32)
